# revision 1
# baseline (speedup 1.0000x reference)
"""Trainium2 Bass kernel for CrossAttention (B=2, Sq=2048, Skv=4096, D=768, H=12).

Sharding: 8 cores = 2 batches x 4 head-groups (3 heads each). Each core
computes its 3 heads' attention and a partial output projection; the host
sums the 4 partial projections per batch and adds bo.

Device data flow (per core, matmul inputs bf16, accumulation fp32):
  - hidden^T / encoder^T shipped pre-transposed; DMAs ordered so the score
    pipeline can start ~20us in (weights, enc^T half, hid^T half, rest).
  - Q^T/K^T computed in [dh(part), seq(free)] layout. Heads h0,h1 stacked on
    partition halves; h2 duplicated on both halves (weights shipped
    duplicated) so every score matmul runs as a pair of concurrent K=64
    row-tiles (full PE array on HW).
  - Scores S^T[kv, q] per 128-kv chunk, grouped into alternating
    [128,1536]/[128,1024] PSUM megas so ScalarE exp amortizes its startup.
  - V projection and the qc1-3 Q^T projections are interleaved into the
    attention stream (aux PSUM slot) so exp starts long before phase-1 ends.
  - PV: V augmented with a ones column (M=65) so PSUM accumulates ctx^T and
    the softmax denominator in one stream; PV issue deferred one mega so
    exp never waits behind PV in the PE queue.
  - Normalization: reciprocal of sums row, gpsimd partition_broadcast, DVE
    multiply; per-head bv added after (softmax rows sum to 1).
  - Output projection: ctx^T is exactly the lhsT needed; partial out to DRAM.
"""

import numpy as np
import ml_dtypes

import concourse.bass as bass
import concourse.bacc as bacc
import concourse.mybir as mybir
from concourse.tile import TileContext
from concourse.bass_utils import run_bass_kernel_spmd

BF16 = mybir.dt.bfloat16
F32 = mybir.dt.float32
AF = mybir.ActivationFunctionType

B, SQ, SKV, D, H, DH = 2, 2048, 4096, 768, 12, 64
HPC = 3          # heads per core
NKC = 6          # contract chunks (768 / 128)
NKV = 32         # kv chunks (4096 / 128)
NQC = 4          # q chunks of 512
QC = 512
P = 128

# wgtA free-dim element offsets (bf16): weights needed first
O_WKAB = 0
O_WQAB = 768
NWA = 1536
# wgtB offsets: the rest
O_WQ2D = 0
O_WK2D = 768
O_WV3 = 1536
O_WOAB = O_WV3 + NKC * 192   # 2688
O_WO2 = O_WOAB + 768
NWB = O_WO2 + 768            # 4224
NE = NKC * SKV               # 24576 encoder^T
NH = NKC * SQ                # 12288 hidden^T

def _mega_layout(n):
    # alternating 3/2-slice megas covering n slices
    sizes = []
    left = n
    while left > 0:
        s = 3 if (len(sizes) % 2 == 0) else 2
        s = min(s, left)
        sizes.append(s)
        left -= s
    base = [0]
    for s in sizes:
        base.append(base[-1] + s)
    return sizes, base

LAST_RESULT = None  # BassKernelResults of the most recent run (for test.py)

_CACHED_NC = None


def _slice_ab(i):
    # phase A slice i (0..63): heads 0/1 interleaved, chunk i//2
    return i % 2, i // 2, i % 2


def _slice_2(i):
    # phase B slice i (0..31): head 2, chunk i, row-half alternating
    return 2, i, i % 2


def _build_nc():
    nc = bacc.Bacc()

    wgtA = nc.declare_dram_parameter("wgtA", [P, NWA], BF16, isOutput=False)
    wgtB = nc.declare_dram_parameter("wgtB", [P, NWB], BF16, isOutput=False)
    biasf = nc.declare_dram_parameter("biasf", [P, 6], F32, isOutput=False)
    eTd = nc.declare_dram_parameter("eT", [P, NE], BF16, isOutput=False)
    hTd = nc.declare_dram_parameter("hT", [P, NH], BF16, isOutput=False)
    out = nc.declare_dram_parameter("out", [SQ, D], F32, isOutput=True)

    with TileContext(nc) as tc:
        with (
            tc.tile_pool(name="persist", bufs=1) as pp,
            tc.tile_pool(name="inb", bufs=1) as ip,
            tc.tile_pool(name="aux", bufs=1, space="PSUM") as auxp,
            tc.tile_pool(name="sT3", bufs=1, space="PSUM") as sT3p,
            tc.tile_pool(name="sT2", bufs=1, space="PSUM") as sT2p,
            tc.tile_pool(name="ctx", bufs=2, space="PSUM") as ctxp,
            tc.tile_pool(name="es", bufs=8) as esp,
            tc.tile_pool(name="fin", bufs=4) as finp,
            tc.tile_pool(name="ost", bufs=2) as ostp,
        ):
            qT_ab = pp.tile([P, SQ], BF16, tag="qT_ab")
            qT_2d = pp.tile([P, SQ], BF16, tag="qT_2d")
            kT_ab = pp.tile([P, SKV], BF16, tag="kT_ab")
            kT_2d = pp.tile([P, SKV], BF16, tag="kT_2d")
            vv = pp.tile([P, NKV, HPC, 65], BF16, tag="vv")
            ctxn_ab = pp.tile([P, SQ], BF16, tag="ctxn_ab")
            ctxn_2 = pp.tile([P, SQ], BF16, tag="ctxn_2")
            wgtA_sb = pp.tile([P, NWA], BF16, tag="wgtA")
            wgtB_sb = pp.tile([P, NWB], BF16, tag="wgtB")
            bias_sb = pp.tile([P, 6], F32, tag="biasf")
            warm = pp.tile([1, 2], F32, tag="warm")
            eT_sb = ip.tile([P, NKC, SKV], BF16, tag="eT")
            hT_sb = ip.tile([P, NKC, SQ], BF16, tag="hT")

            # Warm up the ScalarE exp table while DMAs run.
            nc.vector.memset(warm[0:1, 0:1], 0.0)
            nc.scalar.activation(warm[0:1, 1:2], warm[0:1, 0:1], AF.Exp)
            nc.vector.memset(vv[:, :, :, 64:65], 1.0)

            # Prioritized input loads, all on the sync HWDGE ring (FIFO):
            # first weights for K^T_ab/Q^T_ab, then the first halves of
            # enc^T / hid^T, the remaining weights, then the rest.
            eT_v = eTd.rearrange("p (c q) -> p c q", c=NKC)
            hT_v = hTd.rearrange("p (c q) -> p c q", c=NKC)
            nc.sync.dma_start(out=wgtA_sb, in_=wgtA[:, :])
            nc.sync.dma_start(out=bias_sb, in_=biasf[:, :])
            nc.sync.dma_start(out=eT_sb[:, :, 0:512], in_=eT_v[:, :, 0:512])
            nc.sync.dma_start(out=hT_sb[:, :, 0:512], in_=hT_v[:, :, 0:512])
            nc.sync.dma_start(out=eT_sb[:, :, 512:2048],
                              in_=eT_v[:, :, 512:2048])
            nc.sync.dma_start(out=wgtB_sb, in_=wgtB[:, :])
            nc.sync.dma_start(out=eT_sb[:, :, 2048:SKV],
                              in_=eT_v[:, :, 2048:SKV])
            nc.sync.dma_start(out=hT_sb[:, :, 512:SQ], in_=hT_v[:, :, 512:SQ])

            wk_ab_sb = wgtA_sb[:, O_WKAB:O_WQAB].rearrange("p (c m) -> p c m", c=NKC)
            wq_ab_sb = wgtA_sb[:, O_WQAB:NWA].rearrange("p (c m) -> p c m", c=NKC)
            wq_2d_sb = wgtB_sb[:, O_WQ2D:O_WK2D].rearrange("p (c m) -> p c m", c=NKC)
            wk_2d_sb = wgtB_sb[:, O_WK2D:O_WV3].rearrange("p (c m) -> p c m", c=NKC)
            wv3_sb = wgtB_sb[:, O_WV3:O_WOAB].rearrange("p (c m) -> p c m", c=NKC)
            wo_ab_sb = wgtB_sb[:, O_WOAB:O_WO2]
            wo_2_sb = wgtB_sb[:, O_WO2:NWB]
            bq_ab_sb = bias_sb[:, 0:1]
            bq_2d_sb = bias_sb[:, 1:2]
            bk_ab_sb = bias_sb[:, 2:3]
            bk_2d_sb = bias_sb[:, 3:4]
            bv_ab_sb = bias_sb[:, 4:5]
            bv_2_sb = bias_sb[:, 5:6]

            _scratch_i = [0]

            def _scratch_tile():
                # cycle aux + (pre-attention-idle) score-mega slots
                i = _scratch_i[0] % 3
                _scratch_i[0] += 1
                if i == 0:
                    ps = auxp.tile([P, QC], F32, tag="aux")
                elif i == 1:
                    ps = sT2p.tile([P, 1024], F32, tag="st2")
                else:
                    ps = sT3p.tile([P, 1536], F32, tag="st3")
                return ps

            def proj_cols(dst, w_sb, b_sb, src, q0, q1, scratch=False,
                          on_st2=False):
                # dst[:, q*512...] = (src-chunks contracted with w) + bias
                for q in range(q0, q1):
                    if scratch:
                        ps = _scratch_tile()
                    elif on_st2:
                        ps = sT2p.tile([P, 1024], F32, tag="st2")
                    else:
                        ps = auxp.tile([P, QC], F32, tag="aux")
                    for c in range(NKC):
                        nc.tensor.matmul(
                            ps[:, 0:QC], w_sb[:, c, :],
                            src[:, c, q * QC:(q + 1) * QC],
                            start=(c == 0), stop=(c == NKC - 1),
                        )
                    nc.vector.tensor_scalar_add(
                        dst[:, q * QC:(q + 1) * QC], ps[:, 0:QC], b_sb)

            next_v = [0]

            def emit_v(upto):
                # V projection, two kv-tiles per aux PSUM allocation
                # (disjoint free ranges + one combined copy, so successive
                # MM groups don't serialize on the copy's WAR).
                while next_v[0] < min(upto + 1, NKV):
                    t = next_v[0]
                    ps = auxp.tile([P, QC], F32, tag="aux")
                    for j in range(2):
                        for c in range(NKC):
                            nc.tensor.matmul(
                                ps[:, j * 192:(j + 1) * 192],
                                eT_sb[:, c, (t + j) * 128:(t + j + 1) * 128],
                                wv3_sb[:, c, :],
                                start=(c == 0), stop=(c == NKC - 1),
                            )
                    nc.vector.tensor_copy(
                        vv[:, t:t + 2, :, 0:64],
                        ps[:, 0:384].rearrange("p (t h d) -> p t h d",
                                               t=2, h=HPC),
                    )
                    next_v[0] = t + 2

            def finalize(ctx_t, dst_tile, dst_rows, bv_sb, q):
                # ctx_t[0:64] = unnormalized ctx^T; ctx_t[64] = softmax sums
                rc = finp.tile([1, QC], F32, tag="rc")
                nc.vector.reciprocal(rc, ctx_t[64:65, :])
                bc = finp.tile([64, QC], F32, tag="bc")
                nc.gpsimd.partition_broadcast(bc, rc)
                dst = dst_tile[dst_rows[0]:dst_rows[1], q * QC:(q + 1) * QC]
                nc.vector.tensor_mul(dst, ctx_t[0:64, :], bc)
                nc.vector.tensor_scalar_add(dst, dst, bv_sb)

            # --- pre-attention: first K^T columns, Q^T for qc0 ---
            proj_cols(kT_ab, wk_ab_sb, bk_ab_sb, eT_sb, 0, 2, scratch=True)
            proj_cols(qT_ab, wq_ab_sb, bq_ab_sb, hT_sb, 0, 1, scratch=True)

            def attention_window(q, slice_info, nsl, heads, jit):
                """One attention window: heads' scores+exp+PV for q-chunk q.

                jit: list of thunks; one is drained per mega into the PE
                stream (deferred phase-1 work with deadlines before/inside
                this window).
                """
                qs = slice(q * QC, (q + 1) * QC)
                sizes, base = _mega_layout(nsl)
                ctxs = {}
                for h in heads:
                    ctx_t = ctxp.tile([P, QC], F32, tag="ctx")
                    ctxs[h] = ctx_t

                def pv_mega(k, es_t):
                    emit_v(max(slice_info(base[k] + s)[1]
                               for s in range(sizes[k])))
                    for s in range(sizes[k]):
                        h, c, _ = slice_info(base[k] + s)
                        nc.tensor.matmul(
                            ctxs[h][0:65, :], vv[:, c, h, :],
                            es_t[:, s * QC:(s + 1) * QC],
                            start=(c == 0), stop=(c == NKV - 1),
                        )
                        if c == NKV - 1:
                            if h == 0:
                                finalize(ctxs[0], ctxn_ab, (0, 64),
                                         bv_ab_sb[0:64], q)
                            elif h == 1:
                                finalize(ctxs[1], ctxn_ab, (64, 128),
                                         bv_ab_sb[64:128], q)
                            else:
                                finalize(ctxs[2], ctxn_2, (0, 64),
                                         bv_2_sb[0:64], q)

                prev = None
                for m in range(len(sizes)):
                    if jit:
                        jit.pop(0)()
                    sz = sizes[m]
                    if sz == 3:
                        st = sT3p.tile([P, 1536], F32, tag="st3")
                    else:
                        st = sT2p.tile([P, 1024], F32, tag="st2")
                    for s in range(sz):
                        h, c, rh = slice_info(base[m] + s)
                        kt, qt = (kT_ab, qT_ab) if h < 2 else (kT_2d, qT_2d)
                        r0 = rh * 64
                        nc.tensor.matmul(
                            st[:, s * QC:(s + 1) * QC],
                            kt[r0:r0 + 64, c * 128:(c + 1) * 128],
                            qt[r0:r0 + 64, qs],
                            start=True, stop=True,
                        )
                    es_t = esp.tile([P, sz * QC], BF16,
                                    tag=("es3" if sz == 3 else "es2"))
                    nc.scalar.activation(es_t[:, 0:sz * QC], st[:, 0:sz * QC],
                                         AF.Exp, scale=0.125)
                    if prev is not None:
                        pv_mega(*prev)
                    prev = (m, es_t)
                pv_mega(*prev)
                while jit:
                    jit.pop(0)()

            def proj_qtile(q, t):
                qoff = q * QC + t * 128
                ost = ostp.tile([P, D], F32, tag="ost")
                for n in range(2):
                    ns = slice(n * 384, (n + 1) * 384)
                    pj = ctxp.tile([P, 384], F32, tag="ctx")
                    nc.tensor.matmul(
                        pj, ctxn_ab[:, qoff:qoff + 128],
                        wo_ab_sb[:, ns], start=True, stop=False,
                    )
                    nc.tensor.matmul(
                        pj, ctxn_2[0:64, qoff:qoff + 128],
                        wo_2_sb[0:64, ns], start=False, stop=True,
                    )
                    nc.vector.tensor_copy(ost[:, ns], pj)
                nc.sync.dma_start(out=out[qoff:qoff + 128, :], in_=ost)

            def proj_qc(q):
                for t in range(4):
                    proj_qtile(q, t)

            def pc(dst, w, b, src, q0, on_st2=False):
                return lambda: proj_cols(dst, w, b, src, q0, q0 + 1,
                                         on_st2=on_st2)

            # Phase A: heads h0/h1 for all q-chunks. Remaining K^T_ab columns
            # drain into window 0; K^T_2d / Q^T fill later windows' PE slack.
            jitA0 = [pc(kT_ab, wk_ab_sb, bk_ab_sb, eT_sb, g) for g in range(2, 8)]
            attention_window(0, _slice_ab, 64, (0, 1), jitA0)
            for q in range(1, NQC):
                # q==1: route Q^T around the aux slot (busy with the long
                # V chain from window 0) so window 1 isn't gated on it.
                jitq = [pc(qT_ab, wq_ab_sb, bq_ab_sb, hT_sb, q,
                           on_st2=(q == 1))]
                if q == 1:
                    jitq += [pc(kT_2d, wk_2d_sb, bk_2d_sb, eT_sb, g)
                             for g in range(0, 4)]
                elif q == 2:
                    jitq += [pc(kT_2d, wk_2d_sb, bk_2d_sb, eT_sb, g)
                             for g in range(4, 8)]
                else:
                    jitq += [pc(qT_2d, wq_2d_sb, bq_2d_sb, hT_sb, 0)]
                attention_window(q, _slice_ab, 64, (0, 1), jitq)

            # Phase B: head h2 per q-chunk; the previous chunk's projection
            # drains per-qtile into the next window's PE slack so it never
            # sits ahead of that window's scores in the PE queue.
            for q in range(NQC):
                jitq = []
                if q < NQC - 1:
                    jitq.append(pc(qT_2d, wq_2d_sb, bq_2d_sb, hT_sb, q + 1))
                if q > 0:
                    jitq += [(lambda qq, tt: lambda: proj_qtile(qq, tt))(q - 1, t)
                             for t in range(4)]
                attention_window(q, _slice_2, 32, (2,), jitq)
            proj_qc(NQC - 1)
    nc.finalize()
    return nc


def _bf16(x):
    return np.ascontiguousarray(x.astype(ml_dtypes.bfloat16))


def _pack(w):
    # [768, M] -> [128, 6*M]  (partition-major view of 6 contract chunks)
    m = w.shape[1]
    return w.reshape(NKC, P, m).transpose(1, 0, 2).reshape(P, NKC * m)


def kernel(hidden_states, encoder_hidden_states, Wq, bq, Wk, bk, Wv, bv, Wo, bo):
    global LAST_RESULT, _CACHED_NC
    hidden_states = np.asarray(hidden_states, np.float32)
    encoder_hidden_states = np.asarray(encoder_hidden_states, np.float32)
    Wq, bq = np.asarray(Wq, np.float32), np.asarray(bq, np.float32)
    Wk, bk = np.asarray(Wk, np.float32), np.asarray(bk, np.float32)
    Wv, bv = np.asarray(Wv, np.float32), np.asarray(bv, np.float32)
    Wo, bo = np.asarray(Wo, np.float32), np.asarray(bo, np.float32)

    if _CACHED_NC is None:
        _CACHED_NC = _build_nc()
    nc = _CACHED_NC

    in_maps = []
    for core in range(8):
        b, g = divmod(core, 4)
        h0, h1, h2 = 3 * g, 3 * g + 1, 3 * g + 2
        sl = [slice(DH * h, DH * (h + 1)) for h in (h0, h1, h2)]
        wgtA_np = _bf16(np.concatenate([
            _pack(np.concatenate([Wk[:, sl[0]], Wk[:, sl[1]]], 1)),  # wk_ab
            _pack(np.concatenate([Wq[:, sl[0]], Wq[:, sl[1]]], 1)),  # wq_ab
        ], axis=1))
        wgtB_np = _bf16(np.concatenate([
            _pack(np.concatenate([Wq[:, sl[2]], Wq[:, sl[2]]], 1)),  # wq_2d
            _pack(np.concatenate([Wk[:, sl[2]], Wk[:, sl[2]]], 1)),  # wk_2d
            _pack(np.concatenate([Wv[:, s] for s in sl], 1)),        # wv3
            np.concatenate([Wo[sl[0]], Wo[sl[1]]], 0),               # wo_ab
            np.concatenate([Wo[sl[2]], np.zeros((64, D), np.float32)], 0),
        ], axis=1))
        assert wgtA_np.shape == (P, NWA) and wgtB_np.shape == (P, NWB)
        bias_np = np.stack([
            np.concatenate([bq[sl[0]], bq[sl[1]]]),
            np.concatenate([bq[sl[2]], bq[sl[2]]]),
            np.concatenate([bk[sl[0]], bk[sl[1]]]),
            np.concatenate([bk[sl[2]], bk[sl[2]]]),
            np.concatenate([bv[sl[0]], bv[sl[1]]]),
            np.concatenate([bv[sl[2]], np.zeros(64, np.float32)]),
        ], axis=1).astype(np.float32)
        in_maps.append({
            "wgtA": wgtA_np,
            "wgtB": wgtB_np,
            "biasf": bias_np,
            "eT": _bf16(_pack(encoder_hidden_states[b].T.copy())),
            "hT": _bf16(_pack(hidden_states[b].T.copy())),
        })

    res = run_bass_kernel_spmd(nc, in_maps, list(range(8)))
    LAST_RESULT = res

    outp = np.zeros((B, SQ, D), np.float32)
    for core in range(8):
        b = core // 4
        outp[b] += res.results[core]["out"]
    outp += bo
    return outp

